# revision 10
# baseline (speedup 1.0000x reference)
"""Trainium2 Bass kernel for nn_CrystalDynamics (EGNN message passing).

Data-parallel over crystals: 8 NeuronCores x 8 graphs x 128 nodes each.
Per core: build per-graph 128x128 distance blocks on the PE, extract k=12
nearest neighbours with DVE max8/max_index/match_replace, materialize
per-graph one-hot gather matrices (is_equal against a broadcast index row),
then run the 4 EGNN layers with feature-major activations. The h_src
gather, h_dst broadcast and dist_sq rank-1 term fuse into one PSUM
accumulation per 512-edge chunk; per-edge MLPs stream on the PE in fp32r.
Graph build and layer loops are software-pipelined (emission order is the
per-engine execution order). Outputs return feature-major; the host
transposes during unsharding.
"""

import math
import numpy as np
from contextlib import ExitStack

import concourse.bass as bass
import concourse.tile as tile
import concourse.bacc as bacc
import concourse.mybir as mybir
from concourse.alu_op_type import AluOpType
from concourse.bass_utils import run_bass_kernel_spmd

dt = mybir.dt
AF = mybir.ActivationFunctionType

N_CORES = 8
B = 64
GPC = B // N_CORES   # 8 graphs per core
NPG = 128            # nodes per graph
NPC = GPC * NPG      # 1024 nodes per core
KNN = 12
EPG = NPG * KNN      # 1536 edges per graph
EPC = GPC * EPG      # 12288 edges per core
NODE = 64
TIME = 64
ZDIM = 128
L = 4
CH = 512             # edge chunk size
NCH = EPG // CH      # 3 chunks per graph
TWO_PI = 2.0 * math.pi
NEG_BIG = -1.0e30
GPR = 4              # graphs per packed dist row (rows 0 and 32)

# packed weight order (all [64, 64] f32r)
WNAMES = ["ew1a", "ew1b", "ew1d", "ew2", "cw1", "nw1a", "nw1b", "nw1d", "nw2"]
# packed bias order (all [64] -> bcat rows 0:64)
BNAMES = ["eb1", "eb2", "cb1", "nb1", "nb2"]

_CACHE = {}


def _declare(nc):
    aps = {}

    def inp(name, shape, dtype):
        aps[name] = nc.dram_tensor(name, shape, dtype, kind="ExternalInput").ap()

    def outp(name, shape, dtype):
        aps[name] = nc.dram_tensor(name, shape, dtype, kind="ExternalOutput").ap()

    inp("zT", [ZDIM, NPC], dt.float32r)
    inp("cartT", [3, NPC], dt.float32)
    inp("chi_cat", [NPG, 4 * 3 * GPC], dt.float32r)  # cart hi, lo, -hi, -lo node-major
    inp("wcat", [NODE, len(WNAMES) * L * NODE + 3 * L], dt.float32r)
    # bcat cols: BNAMES x L, embb, tb2, tb1(full 128), pidx(full 128), sqrt_eps
    inp("bcat", [ZDIM, len(BNAMES) * L + 5], dt.float32)
    inp("e13", [33, L * NODE + 3], dt.float32r)      # ew1c x4 + ones13 on rows 0,32
    inp("rowcat", [1, 2 * TIME + NPG + 2 * GPC], dt.float32)
    inp("onesr_r", [1, NPG], dt.float32r)
    inp("ones3", [3, 1], dt.float32)
    inp("negdiag", [NPG, NPG], dt.float32)
    inp("gdst", [NPG, EPG], dt.float32r)
    inp("tw1", [TIME, 2 * TIME], dt.float32)
    inp("tw2", [2 * TIME, TIME], dt.float32)
    inp("embw", [ZDIM, NODE], dt.float32r)

    outp("o_coordT", [3, NPC], dt.float32)
    outp("o_hT", [NODE, NPC], dt.float32r)
    if _CACHE.get("debug"):
        outp("o_udiff", [3, EPC], dt.float32)
        outp("o_miT", [NODE, NPC], dt.float32r)
    return aps


def _emit(nc, tc, ctx, aps):
    cst = ctx.enter_context(tc.tile_pool(name="cst", bufs=1))
    big = ctx.enter_context(tc.tile_pool(name="big", bufs=1))
    wrk = ctx.enter_context(tc.tile_pool(name="wrk", bufs=2))
    drp = ctx.enter_context(tc.tile_pool(name="drp", bufs=3, space="DRAM"))
    ps = ctx.enter_context(tc.tile_pool(name="ps", bufs=5, space="PSUM"))

    def psum(shape, tag="ps", bufs=None):
        return ps.tile(shape, dt.float32, tag=tag, name=tag, bufs=bufs)

    def load(name, pool=cst):
        ap = aps[name]
        t = pool.tile(list(ap.shape), ap.dtype, tag=name, name=name)
        nc.sync.dma_start(t[:], ap[:])
        return t

    cartT = load("cartT")
    chi = load("chi_cat")
    wcat = load("wcat")
    bcat = load("bcat")
    e13 = load("e13")
    rowcat = load("rowcat")
    onesr_r = load("onesr_r")
    ones3 = load("ones3")
    negdiag = load("negdiag")
    gdst = load("gdst", pool=big)
    tw1 = load("tw1")
    tw2 = load("tw2")
    embw = load("embw")

    # packed views
    W = {}
    for l in range(L):
        for i, nm in enumerate(WNAMES):
            c0 = (l * len(WNAMES) + i) * NODE
            W[(nm, l)] = wcat[:, c0:c0 + NODE]
    c0 = len(WNAMES) * L * NODE
    for l in range(L):
        W[("cw2r3", l)] = wcat[:, c0 + 3 * l:c0 + 3 * l + 3]
    Bv = {}
    for l in range(L):
        for i, nm in enumerate(BNAMES):
            Bv[(nm, l)] = bcat[0:NODE, l * len(BNAMES) + i:l * len(BNAMES) + i + 1]
    nbi = len(BNAMES) * L
    Bv["embb"] = bcat[0:NODE, nbi:nbi + 1]
    Bv["tb2"] = bcat[0:NODE, nbi + 1:nbi + 2]
    Bv["tb1"] = bcat[:, nbi + 2:nbi + 3]
    pidxc = bcat[:, nbi + 3:nbi + 4]
    Bv["sqrt_eps"] = bcat[0:3, nbi + 4:nbi + 5]

    def ew1c2(l):
        return e13[:, l * NODE:(l + 1) * NODE]
    ones13r2 = e13[:, L * NODE:L * NODE + 3]

    def chi_s(kind, g):
        # kind: 0 hi, 1 lo, 2 -hi, 3 -lo
        c0 = kind * 3 * GPC + 3 * g
        return chi[:, c0:c0 + 3]

    # rowcat views
    freqext = rowcat[:, 0:TIME]
    halfpi = rowcat[:, TIME:2 * TIME]
    onesr = rowcat[:, 2 * TIME:2 * TIME + NPG]
    t_row = rowcat[:, 2 * TIME + NPG:2 * TIME + NPG + GPC]
    ones1g = rowcat[:, 2 * TIME + NPG + GPC:2 * TIME + NPG + 2 * GPC]

    hT = big.tile([NODE, NPC], dt.float32r, tag="hT", name="hT")
    miT = big.tile([NODE, NPC], dt.float32r, tag="miT", name="miT")
    coordT = big.tile([3, NPC], dt.float32, tag="coordT", name="coordT")
    nc.vector.memset(coordT[:], 0.0)
    G = big.tile([NPG, EPC], dt.float32r, tag="G", name="G")
    dist2 = big.tile([33, GPR * EPG], dt.float32r, tag="dist2", name="dist2")
    udiff = big.tile([3, EPC], dt.bfloat16, tag="udiff", name="udiff")

    def dist_slice(g, lo, n):
        return dist2[32 * (g // GPR):32 * (g // GPR) + 1,
                     (g % GPR) * EPG + lo:(g % GPR) * EPG + lo + n]

    # ---- time embedding ----
    p_ang = psum([TIME, GPC])
    nc.tensor.matmul(p_ang[:], freqext, t_row, start=True, stop=False)
    nc.tensor.matmul(p_ang[:], halfpi, ones1g, start=False, stop=True)
    ang = cst.tile([TIME, GPC], dt.float32, tag="ang", name="ang")
    nc.vector.tensor_copy(ang[:], p_ang[:])
    xs = cst.tile([TIME, GPC], dt.float32, tag="xs", name="xs")
    nc.vector.tensor_scalar(xs[:], ang[:], float(1.0 / TWO_PI), None, op0=AluOpType.mult)
    ki = cst.tile([TIME, GPC], dt.int32, tag="ki", name="ki")
    nc.vector.tensor_copy(ki[:], xs[:])
    kf = cst.tile([TIME, GPC], dt.float32, tag="kf", name="kf")
    nc.vector.tensor_copy(kf[:], ki[:])
    red = cst.tile([TIME, GPC], dt.float32, tag="red", name="red")
    c1 = float(np.float32(6.28125))
    c2 = float(np.float32(TWO_PI - 6.28125))
    c3 = float(np.float32(TWO_PI - 6.28125 - float(np.float32(TWO_PI - 6.28125))))
    nc.vector.cody_waite_cascade(red[:], ang[:], kf[:], c1, c2, c3)
    te0 = cst.tile([TIME, GPC], dt.float32, tag="te0", name="te0")
    nc.scalar.activation(te0[:], red[:], AF.Sin)
    p_t1 = psum([2 * TIME, GPC])
    nc.tensor.matmul(p_t1[:], tw1[:], te0[:])
    s_t1 = cst.tile([2 * TIME, GPC], dt.float32, tag="s_t1", name="s_t1")
    nc.scalar.activation(s_t1[:], p_t1[:], AF.Silu, bias=Bv["tb1"])
    p_t2 = psum([TIME, GPC])
    nc.tensor.matmul(p_t2[:], tw2[:], s_t1[:])
    teT = cst.tile([TIME, GPC], dt.float32r, tag="teT", name="teT")
    nc.vector.tensor_scalar(teT[:], p_t2[:], Bv["tb2"], None, op0=AluOpType.add)

    te_eb, te_nb = [], []
    for l in range(L):
        p_b = psum([NODE, GPC])
        nc.tensor.matmul(p_b[:], W[("ew1d", l)], teT[:])
        eb = cst.tile([NODE, GPC], dt.float32, tag=f"te_eb{l}", name=f"te_eb{l}")
        nc.vector.tensor_scalar(eb[:], p_b[:], Bv[("eb1", l)], None, op0=AluOpType.add)
        te_eb.append(eb)
        p_b2 = psum([NODE, GPC])
        nc.tensor.matmul(p_b2[:], W[("nw1d", l)], teT[:])
        nbt = cst.tile([NODE, GPC], dt.float32, tag=f"te_nb{l}", name=f"te_nb{l}")
        nc.vector.tensor_scalar(nbt[:], p_b2[:], Bv[("nb1", l)], None, op0=AluOpType.add)
        te_nb.append(nbt)

    # ---- node embedding (zT staged through G's SBUF space, overwritten later) ----
    zT = G[:, 0:NPC]
    nc.sync.dma_start(zT, aps["zT"][:])
    for c in range(NPC // CH):
        p_h = psum([NODE, CH])
        nc.tensor.matmul(p_h[:], embw[:], zT[:, c * CH:(c + 1) * CH])
        nc.vector.tensor_scalar(hT[:, c * CH:(c + 1) * CH], p_h[:], Bv["embb"],
                                None, op0=AluOpType.add)

    # ---- graph build (software-pipelined over graphs) ----
    GS = [dict() for _ in range(GPC)]

    def b0(g):
        st = GS[g]
        cg = cartT[:, g * NPG:(g + 1) * NPG]
        c2t = wrk.tile([3, NPG], dt.float32, tag="c2t", name="c2t", bufs=2)
        nc.scalar.square(c2t[:], cg)
        c3x2 = wrk.tile([3, NPG], dt.float32, tag="c3x2", name="c3x2", bufs=2)
        nc.scalar.mul(c3x2[:], cg, 2.0)
        p_sq = psum([1, NPG], tag="p_sq", bufs=1)
        nc.tensor.matmul(p_sq[:], ones3[:], c2t[:])
        sqr = wrk.tile([1, NPG], dt.float32, tag="sqr", name="sqr", bufs=2)
        nc.vector.tensor_copy(sqr[:], p_sq[:])
        nsqr = wrk.tile([1, NPG], dt.float32, tag="nsqr", name="nsqr", bufs=2)
        nc.scalar.mul(nsqr[:], sqr[:], -1.0)
        negr = wrk.tile([1, NPG], dt.float32, tag="negr", name="negr", bufs=2)
        nc.scalar.mul(negr[:], onesr, -1.0)
        p_D = psum([NPG, NPG], tag="p_D", bufs=2)
        nc.tensor.matmul(p_D[:], c3x2[:], cg, start=True, stop=False)
        nc.tensor.matmul(p_D[:], negr[:], sqr[:], start=False, stop=False)
        nc.tensor.matmul(p_D[:], nsqr[:], onesr, start=False, stop=True)
        Bt = wrk.tile([NPG, NPG], dt.float32, tag="Bt", name="Bt", bufs=2)
        nc.vector.tensor_add(Bt[:], p_D[:], negdiag[:])
        st["Bt"] = Bt

    def b1(g):
        st = GS[g]
        Bt = st.pop("Bt")
        mx1 = wrk.tile([NPG, 8], dt.float32, tag="mx1", name="mx1", bufs=2)
        ix1 = wrk.tile([NPG, 8], dt.uint32, tag="ix1", name="ix1", bufs=2)
        nc.vector.max(mx1[:], Bt[:])
        nc.vector.max_index(ix1[:], mx1[:], Bt[:])
        B2t = wrk.tile([NPG, NPG], dt.float32, tag="B2t", name="B2t", bufs=2)
        nc.vector.match_replace(B2t[:], mx1[:], Bt[:], NEG_BIG)
        mx2 = wrk.tile([NPG, 8], dt.float32, tag="mx2", name="mx2", bufs=2)
        ix2 = wrk.tile([NPG, 8], dt.uint32, tag="ix2", name="ix2", bufs=2)
        nc.vector.max(mx2[:], B2t[:])
        nc.vector.max_index(ix2[:], mx2[:], B2t[:])
        idxf = wrk.tile([NPG, KNN], dt.float32r, tag="idxf", name="idxf", bufs=3)
        nc.vector.tensor_copy(idxf[:, 0:8], ix1[:])
        nc.vector.tensor_copy(idxf[:, 8:12], ix2[:, 0:4])
        dnm = wrk.tile([NPG, KNN], dt.float32r, tag="dnm", name="dnm", bufs=3)
        nc.scalar.mul(dnm[:, 0:8], mx1[:], -1.0)
        nc.scalar.mul(dnm[:, 8:12], mx2[:, 0:4], -1.0)
        st["idxf"], st["dnm"] = idxf, dnm

    def b2(g):
        st = GS[g]
        scr_i = drp.tile([NPG, KNN], dt.float32r, tag="scr_i", name="scr_i", bufs=3)
        scr_d = drp.tile([NPG, KNN], dt.float32r, tag="scr_d", name="scr_d", bufs=3)
        nc.sync.dma_start(scr_i[:], st.pop("idxf")[:])
        nc.scalar.dma_start(scr_d[:], st.pop("dnm")[:])
        idx_row = wrk.tile([1, EPG], dt.float32r, tag="idx_row", name="idx_row", bufs=2)
        nc.sync.dma_start(idx_row[:], scr_i[:].rearrange("p k -> (p k)")[None, :])
        nc.scalar.dma_start(dist_slice(g, 0, EPG),
                            scr_d[:].rearrange("p k -> (p k)")[None, :])
        st["idx_row"] = idx_row

    def b3(g):
        st = GS[g]
        idx_row = st.pop("idx_row")
        for c in range(NCH):
            e0 = g * EPG + c * CH
            p_bc = psum([NPG, CH])
            nc.tensor.matmul(p_bc[:], onesr_r[:], idx_row[:, c * CH:(c + 1) * CH])
            nc.vector.tensor_scalar(G[:, e0:e0 + CH], p_bc[:], pidxc, None,
                                    op0=AluOpType.is_equal)

    def b4(g):
        j = 32 * (g // GPR)
        for c in range(NCH):
            e0 = g * EPG + c * CH
            p_df = psum([3, CH])
            nc.tensor.matmul(p_df[:], chi_s(0, g), G[:, e0:e0 + CH],
                             start=True, stop=False)
            nc.tensor.matmul(p_df[:], chi_s(1, g), G[:, e0:e0 + CH],
                             start=False, stop=False)
            nc.tensor.matmul(p_df[:], chi_s(2, g), gdst[:, c * CH:(c + 1) * CH],
                             start=False, stop=False)
            nc.tensor.matmul(p_df[:], chi_s(3, g), gdst[:, c * CH:(c + 1) * CH],
                             start=False, stop=True)
            p_d3 = psum([3, CH])
            nc.tensor.matmul(p_d3[:], ones13r2[j:j + 1, :], dist_slice(g, c * CH, CH))
            d3 = wrk.tile([3, CH], dt.float32, tag="d3", name="d3", bufs=2)
            nc.scalar.activation(d3[:], p_d3[:], AF.Sqrt, bias=Bv["sqrt_eps"])
            r3 = wrk.tile([3, CH], dt.float32, tag="r3", name="r3", bufs=2)
            nc.vector.reciprocal(r3[:], d3[:])
            nc.vector.tensor_tensor(udiff[:, e0:e0 + CH], p_df[:], r3[:],
                                    op=AluOpType.mult)

    bstages = [b0, b1, b2, b3, b4]
    for i in range(GPC + len(bstages) - 1):
        for sidx, fn in enumerate(bstages):
            k = i - sidx
            if 0 <= k < GPC:
                fn(k)

    # ---- layers (software-pipelined over 512-edge units) ----
    for l in range(L):
        pgs, qgs = [], []
        for g in range(GPC):
            hg = hT[:, g * NPG:(g + 1) * NPG]
            p_p = psum([NPG, NODE])
            nc.tensor.matmul(p_p[:], hg, W[("ew1a", l)])
            pg = wrk.tile([NPG, NODE], dt.float32r, tag="pg", name="pg", bufs=GPC + 1)
            nc.vector.tensor_copy(pg[:], p_p[:])
            p_q = psum([NPG, NODE])
            nc.tensor.matmul(p_q[:], hg, W[("ew1b", l)])
            qg = wrk.tile([NPG, NODE], dt.float32r, tag="qg", name="qg", bufs=GPC + 1)
            nc.vector.tensor_copy(qg[:], p_q[:])
            pgs.append(pg); qgs.append(qg)

        units = [(g, c) for g in range(GPC) for c in range(NCH)]
        S = {}
        mTs, csTs = {}, {}

        def stage0(u, l=l, pgs=pgs, qgs=qgs, S=S):
            g, c = u
            j = 32 * (g // GPR)
            e0 = g * EPG + c * CH
            p_e = psum([NODE, CH])
            nc.tensor.matmul(p_e[:], pgs[g][:], G[:, e0:e0 + CH],
                             start=True, stop=False)
            nc.tensor.matmul(p_e[:], qgs[g][:], gdst[:, c * CH:(c + 1) * CH],
                             start=False, stop=False)
            nc.tensor.matmul(p_e[:], ew1c2(l)[j:j + 1, :],
                             dist_slice(g, c * CH, CH), start=False, stop=True)
            h1 = wrk.tile([NODE, CH], dt.float32r, tag="h1", name="h1", bufs=3)
            nc.scalar.activation(h1[:], p_e[:], AF.Silu, bias=te_eb[l][:, g:g + 1])
            S[u] = {"h1": h1}

        def stage1(u, l=l, S=S, mTs=mTs):
            g, c = u
            if c == 0:
                mTs[g] = wrk.tile([NODE, EPG], dt.float32r, tag="mT", name="mT", bufs=2)
            p_m = psum([NODE, CH])
            nc.tensor.matmul(p_m[:], W[("ew2", l)], S[u]["h1"][:])
            nc.scalar.activation(mTs[g][:, c * CH:(c + 1) * CH], p_m[:], AF.Silu,
                                 bias=Bv[("eb2", l)])

        def stage2(u, l=l, S=S, mTs=mTs):
            g, c = u
            p_c = psum([NODE, CH])
            nc.tensor.matmul(p_c[:], W[("cw1", l)], mTs[g][:, c * CH:(c + 1) * CH])
            ch1 = wrk.tile([NODE, CH], dt.float32r, tag="ch1", name="ch1", bufs=3)
            nc.scalar.activation(ch1[:], p_c[:], AF.Silu, bias=Bv[("cb1", l)])
            S[u]["ch1"] = ch1

        def stage3(u, l=l, S=S, mTs=mTs, csTs=csTs):
            g, c = u
            e0 = g * EPG + c * CH
            if c == 0:
                csTs[g] = wrk.tile([3, EPG], dt.float32, tag="csT", name="csT", bufs=2)
            p_g3 = psum([3, CH])
            nc.tensor.matmul(p_g3[:], W[("cw2r3", l)], S[u]["ch1"][:])
            nc.vector.tensor_tensor(csTs[g][:, c * CH:(c + 1) * CH], p_g3[:],
                                    udiff[:, e0:e0 + CH], op=AluOpType.mult)
            S.pop(u)
            if c == NCH - 1:
                nc.vector.tensor_reduce(miT[:, g * NPG:(g + 1) * NPG],
                                        mTs[g][:].rearrange("p (n k) -> p n k", k=KNN),
                                        axis=mybir.AxisListType.X, op=AluOpType.add)
                ctmp = wrk.tile([3, NPG], dt.float32, tag="ctmp", name="ctmp", bufs=2)
                nc.vector.tensor_reduce(ctmp[:],
                                        csTs[g][:].rearrange("p (n k) -> p n k", k=KNN),
                                        axis=mybir.AxisListType.X, op=AluOpType.add)
                nc.vector.tensor_add(coordT[:, g * NPG:(g + 1) * NPG],
                                     coordT[:, g * NPG:(g + 1) * NPG], ctmp[:])

        stages = [stage0, stage1, stage2, stage3]
        n = len(units)
        for i in range(n + len(stages) - 1):
            for sidx, fn in enumerate(stages):
                k = i - sidx
                if 0 <= k < n:
                    fn(units[k])

        for c in range(NPC // CH):
            n0 = c * CH
            p_n = psum([NODE, CH])
            nc.tensor.matmul(p_n[:], W[("nw1a", l)], hT[:, n0:n0 + CH],
                             start=True, stop=False)
            nc.tensor.matmul(p_n[:], W[("nw1b", l)], miT[:, n0:n0 + CH],
                             start=False, stop=True)
            sn = wrk.tile([NODE, CH], dt.float32r, tag="sn", name="sn", bufs=2)
            for gg in range(CH // NPG):
                g = (n0 + gg * NPG) // NPG
                nc.scalar.activation(sn[:, gg * NPG:(gg + 1) * NPG],
                                     p_n[:, gg * NPG:(gg + 1) * NPG], AF.Silu,
                                     bias=te_nb[l][:, g:g + 1])
            p_n2 = psum([NODE, CH])
            nc.tensor.matmul(p_n2[:], W[("nw2", l)], sn[:])
            nc.vector.scalar_tensor_tensor(hT[:, n0:n0 + CH], p_n2[:], Bv[("nb2", l)],
                                           hT[:, n0:n0 + CH],
                                           op0=AluOpType.add, op1=AluOpType.add)

    nc.sync.dma_start(aps["o_coordT"][:], coordT[:])
    nc.sync.dma_start(aps["o_hT"][:], hT[:])
    if _CACHE.get("debug"):
        nc.sync.dma_start(aps["o_udiff"][:], udiff[:])
        nc.sync.dma_start(aps["o_miT"][:], miT[:])


def _build():
    if "nc" in _CACHE:
        return _CACHE["nc"]
    nc = bacc.Bacc("TRN2", target_bir_lowering=False, debug=False,
                   enable_asserts=True, num_devices=N_CORES)
    aps = _declare(nc)
    with tile.TileContext(nc, trace_sim=False) as tc:
        with ExitStack() as ctx:
            with nc.allow_low_precision(reason="fp32r storage is fp32-width"):
                _emit(nc, tc, ctx, aps)
    nc.compile()
    _CACHE["nc"] = nc
    return nc


def _host_inputs(z_nodes, t, cart_coords, batch_indices, params):
    z = np.asarray(z_nodes, np.float32)
    tt = np.asarray(t, np.float32)
    cart = np.asarray(cart_coords, np.float32)

    half = TIME // 2
    freqs = np.exp(np.arange(half, dtype=np.float32) * (-math.log(10000.0) / (half - 1)))

    wcat = np.zeros((NODE, len(WNAMES) * L * NODE + 3 * L), np.float32)
    for l, lp in enumerate(params["layers"]):
        ew1 = np.asarray(lp["edge_w1"], np.float32)
        nw1 = np.asarray(lp["node_w1"], np.float32)
        mats = {
            "ew1a": ew1[0:NODE], "ew1b": ew1[NODE:2 * NODE], "ew1d": ew1[2 * NODE + 1:],
            "ew2": np.asarray(lp["edge_w2"], np.float32),
            "cw1": np.asarray(lp["coord_w1"], np.float32),
            "nw1a": nw1[0:NODE], "nw1b": nw1[NODE:2 * NODE], "nw1d": nw1[2 * NODE:],
            "nw2": np.asarray(lp["node_w2"], np.float32),
        }
        for i, nm in enumerate(WNAMES):
            c0 = (l * len(WNAMES) + i) * NODE
            wcat[:, c0:c0 + NODE] = mats[nm]
    c0 = len(WNAMES) * L * NODE
    for l, lp in enumerate(params["layers"]):
        cw2 = np.asarray(lp["coord_w2"], np.float32)
        wcat[:, c0 + 3 * l:c0 + 3 * l + 3] = np.repeat(cw2, 3, axis=1)

    bcat = np.zeros((ZDIM, len(BNAMES) * L + 5), np.float32)
    for l, lp in enumerate(params["layers"]):
        vals = {
            "eb1": lp["edge_b1"], "eb2": lp["edge_b2"], "cb1": lp["coord_b1"],
            "nb1": lp["node_b1"], "nb2": lp["node_b2"],
        }
        for i, nm in enumerate(BNAMES):
            bcat[0:NODE, l * len(BNAMES) + i] = np.asarray(vals[nm], np.float32)
    nbi = len(BNAMES) * L
    bcat[0:NODE, nbi] = np.asarray(params["emb_b"], np.float32)
    bcat[0:NODE, nbi + 1] = np.asarray(params["time_b2"], np.float32)
    bcat[:, nbi + 2] = np.asarray(params["time_b1"], np.float32)
    bcat[:, nbi + 3] = np.arange(ZDIM, dtype=np.float32)
    bcat[0:3, nbi + 4] = 1e-8

    e13 = np.zeros((33, L * NODE + 3), np.float32)
    for l, lp in enumerate(params["layers"]):
        w1c = np.asarray(lp["edge_w1"], np.float32)[2 * NODE]
        e13[0, l * NODE:(l + 1) * NODE] = w1c
        e13[32, l * NODE:(l + 1) * NODE] = w1c
    e13[0, L * NODE:] = 1.0
    e13[32, L * NODE:] = 1.0

    rowcat_shared = np.zeros((1, 2 * TIME + NPG + 2 * GPC), np.float32)
    rowcat_shared[0, 0:half] = freqs
    rowcat_shared[0, half:TIME] = freqs
    rowcat_shared[0, TIME + half:2 * TIME] = np.pi / 2
    rowcat_shared[0, 2 * TIME:2 * TIME + NPG] = 1.0
    rowcat_shared[0, 2 * TIME + NPG + GPC:] = 1.0

    shared = {
        "wcat": wcat,
        "bcat": bcat,
        "e13": e13,
        "onesr_r": np.ones((1, NPG), np.float32),
        "ones3": np.ones((3, 1), np.float32),
        "negdiag": (np.eye(NPG) * NEG_BIG).astype(np.float32),
        "gdst": np.kron(np.eye(NPG, dtype=np.float32), np.ones((1, KNN), np.float32)),
        "tw1": np.asarray(params["time_w1"], np.float32),
        "tw2": np.asarray(params["time_w2"], np.float32),
        "embw": np.asarray(params["emb_w"], np.float32),
    }

    in_maps = []
    for cix in range(N_CORES):
        n0 = cix * NPC
        zc = z[n0:n0 + NPC]
        cc = cart[n0:n0 + NPC]
        cnm = np.zeros((NPG, 3 * GPC), np.float32)
        for g in range(GPC):
            cnm[:, 3 * g:3 * g + 3] = cc[g * NPG:(g + 1) * NPG]
        cnm_hi = (cnm.view(np.uint32) & np.uint32(0xFFFF0000)).view(np.float32)
        cnm_lo = cnm - cnm_hi
        rc = rowcat_shared.copy()
        rc[0, 2 * TIME + NPG:2 * TIME + NPG + GPC] = tt[cix * GPC:(cix + 1) * GPC]
        m = dict(shared)
        m["rowcat"] = rc
        m["zT"] = np.ascontiguousarray(zc.T)
        m["cartT"] = np.ascontiguousarray(cc.T)
        m["chi_cat"] = np.concatenate([cnm_hi, cnm_lo, -cnm_hi, -cnm_lo], axis=1)
        in_maps.append(m)
    return in_maps


def kernel(z_nodes, t, cart_coords, batch_indices, params):
    nc = _build()
    in_maps = _host_inputs(z_nodes, t, cart_coords, batch_indices, params)
    res = run_bass_kernel_spmd(nc, in_maps, list(range(N_CORES))).results
    coord = np.concatenate([np.ascontiguousarray(r["o_coordT"].T) for r in res], axis=0)
    h = np.concatenate([np.ascontiguousarray(r["o_hT"].T) for r in res], axis=0)
    return coord.astype(np.float32), h.astype(np.float32)


# revision 13
# speedup vs baseline: 2370.2752x; 2370.2752x over previous
"""Trainium2 Bass kernel for nn_CrystalDynamics (EGNN message passing).

Data-parallel over crystals: 8 NeuronCores x 8 graphs x 128 nodes each.
Per core: build per-graph 128x128 distance blocks on the PE, extract k=12
nearest neighbours with DVE max8/max_index/match_replace, materialize
per-graph one-hot gather matrices (is_equal against a broadcast index row),
then run the 4 EGNN layers with feature-major activations. The h_src
gather, h_dst broadcast and dist_sq rank-1 term fuse into one PSUM
accumulation per 512-edge chunk; per-edge MLPs stream on the PE in fp32r.
Graph build and layer loops are software-pipelined (emission order is the
per-engine execution order). Outputs return feature-major; the host
transposes during unsharding.
"""

import math
import numpy as np
from contextlib import ExitStack

import concourse.bass as bass
import concourse.tile as tile
import concourse.bacc as bacc
import concourse.mybir as mybir
from concourse.alu_op_type import AluOpType
from concourse.bass_utils import run_bass_kernel_spmd

dt = mybir.dt
AF = mybir.ActivationFunctionType

N_CORES = 8
B = 64
GPC = B // N_CORES   # 8 graphs per core
NPG = 128            # nodes per graph
NPC = GPC * NPG      # 1024 nodes per core
KNN = 12
EPG = NPG * KNN      # 1536 edges per graph
EPC = GPC * EPG      # 12288 edges per core
NODE = 64
TIME = 64
ZDIM = 128
L = 4
CH = 512             # edge chunk size
NCH = EPG // CH      # 3 chunks per graph
TWO_PI = 2.0 * math.pi
NEG_BIG = -1.0e30
GPR = 4              # graphs per packed dist row (rows 0 and 32)

# packed weight order (all [64, 64] f32r)
WNAMES = ["ew1a", "ew1b", "ew1d", "ew2", "cw1", "nw1a", "nw1b", "nw1d", "nw2"]
# packed bias order (all [64] -> bcat rows 0:64)
BNAMES = ["eb1", "eb2", "cb1", "nb1", "nb2"]

_CACHE = {}


def _declare(nc):
    aps = {}

    def inp(name, shape, dtype):
        aps[name] = nc.dram_tensor(name, shape, dtype, kind="ExternalInput").ap()

    def outp(name, shape, dtype):
        aps[name] = nc.dram_tensor(name, shape, dtype, kind="ExternalOutput").ap()

    inp("zT", [ZDIM, NPC], dt.float32r)
    inp("cartT", [3, NPC], dt.float32)
    inp("chi_cat", [NPG, 4 * 3 * GPC], dt.float32r)  # cart hi, lo, -hi, -lo node-major
    inp("wcat", [NODE, len(WNAMES) * L * NODE + 3 * L], dt.float32r)
    # bcat cols: BNAMES x L, embb, tb2, tb1(full 128), pidx(full 128), sqrt_eps
    inp("bcat", [ZDIM, len(BNAMES) * L + 5], dt.float32)
    inp("e13", [33, L * NODE + 3], dt.float32r)      # ew1c x4 + ones13 on rows 0,32
    inp("rowcat", [1, 2 * TIME + NPG + 2 * GPC], dt.float32)
    inp("onesr_r", [1, NPG], dt.float32r)
    inp("ones3", [3, 1], dt.float32)
    inp("negdiag", [NPG, NPG], dt.float32)
    inp("gdst", [NPG, EPG], dt.float32r)
    inp("tw1", [TIME, 2 * TIME], dt.float32)
    inp("tw2", [2 * TIME, TIME], dt.float32)
    inp("embw", [ZDIM, NODE], dt.float32r)

    outp("o_coordT", [3, NPC], dt.float32)
    outp("o_hT", [NODE, NPC], dt.float32r)
    if _CACHE.get("debug"):
        outp("o_udiff", [3, EPC], dt.float32)
        outp("o_miT", [NODE, NPC], dt.float32r)
    return aps


def _emit(nc, tc, ctx, aps):
    cst = ctx.enter_context(tc.tile_pool(name="cst", bufs=1))
    big = ctx.enter_context(tc.tile_pool(name="big", bufs=1))
    wrk = ctx.enter_context(tc.tile_pool(name="wrk", bufs=2))
    drp = ctx.enter_context(tc.tile_pool(name="drp", bufs=3, space="DRAM"))
    ps = ctx.enter_context(tc.tile_pool(name="ps", bufs=5, space="PSUM"))

    def psum(shape, tag="ps", bufs=None):
        return ps.tile(shape, dt.float32, tag=tag, name=tag, bufs=bufs)

    def load(name, pool=cst):
        ap = aps[name]
        t = pool.tile(list(ap.shape), ap.dtype, tag=name, name=name)
        nc.sync.dma_start(t[:], ap[:])
        return t

    cartT = load("cartT")
    chi = load("chi_cat")
    wcat = load("wcat")
    bcat = load("bcat")
    e13 = load("e13")
    rowcat = load("rowcat")
    onesr_r = load("onesr_r")
    ones3 = load("ones3")
    negdiag = load("negdiag")
    gdst = load("gdst", pool=big)
    tw1 = load("tw1")
    tw2 = load("tw2")
    embw = load("embw")

    # packed views
    W = {}
    for l in range(L):
        for i, nm in enumerate(WNAMES):
            c0 = (l * len(WNAMES) + i) * NODE
            W[(nm, l)] = wcat[:, c0:c0 + NODE]
    c0 = len(WNAMES) * L * NODE
    for l in range(L):
        W[("cw2r3", l)] = wcat[:, c0 + 3 * l:c0 + 3 * l + 3]
    Bv = {}
    for l in range(L):
        for i, nm in enumerate(BNAMES):
            Bv[(nm, l)] = bcat[0:NODE, l * len(BNAMES) + i:l * len(BNAMES) + i + 1]
    nbi = len(BNAMES) * L
    Bv["embb"] = bcat[0:NODE, nbi:nbi + 1]
    Bv["tb2"] = bcat[0:NODE, nbi + 1:nbi + 2]
    Bv["tb1"] = bcat[:, nbi + 2:nbi + 3]
    pidxc = bcat[:, nbi + 3:nbi + 4]
    Bv["sqrt_eps"] = bcat[0:3, nbi + 4:nbi + 5]

    def ew1c2(l):
        return e13[:, l * NODE:(l + 1) * NODE]
    ones13r2 = e13[:, L * NODE:L * NODE + 3]

    def chi_s(kind, g):
        # kind: 0 hi, 1 lo, 2 -hi, 3 -lo
        c0 = kind * 3 * GPC + 3 * g
        return chi[:, c0:c0 + 3]

    # rowcat views
    freqext = rowcat[:, 0:TIME]
    halfpi = rowcat[:, TIME:2 * TIME]
    onesr = rowcat[:, 2 * TIME:2 * TIME + NPG]
    t_row = rowcat[:, 2 * TIME + NPG:2 * TIME + NPG + GPC]
    ones1g = rowcat[:, 2 * TIME + NPG + GPC:2 * TIME + NPG + 2 * GPC]

    hT = big.tile([NODE, NPC], dt.float32r, tag="hT", name="hT")
    miT = big.tile([NODE, NPC], dt.float32r, tag="miT", name="miT")
    coordT = big.tile([3, NPC], dt.float32, tag="coordT", name="coordT")
    nc.vector.memset(coordT[:], 0.0)
    G = big.tile([NPG, EPC], dt.float32r, tag="G", name="G")
    dist2 = big.tile([33, GPR * EPG], dt.float32r, tag="dist2", name="dist2")
    udiff = big.tile([3, EPC], dt.bfloat16, tag="udiff", name="udiff")

    def dist_slice(g, lo, n):
        return dist2[32 * (g // GPR):32 * (g // GPR) + 1,
                     (g % GPR) * EPG + lo:(g % GPR) * EPG + lo + n]

    # ---- time embedding ----
    p_ang = psum([TIME, GPC])
    nc.tensor.matmul(p_ang[:], freqext, t_row, start=True, stop=False)
    nc.tensor.matmul(p_ang[:], halfpi, ones1g, start=False, stop=True)
    ang = cst.tile([TIME, GPC], dt.float32, tag="ang", name="ang")
    nc.vector.tensor_copy(ang[:], p_ang[:])
    xs = cst.tile([TIME, GPC], dt.float32, tag="xs", name="xs")
    nc.vector.tensor_scalar(xs[:], ang[:], float(1.0 / TWO_PI), None, op0=AluOpType.mult)
    ki = cst.tile([TIME, GPC], dt.int32, tag="ki", name="ki")
    nc.vector.tensor_copy(ki[:], xs[:])
    kf = cst.tile([TIME, GPC], dt.float32, tag="kf", name="kf")
    nc.vector.tensor_copy(kf[:], ki[:])
    red = cst.tile([TIME, GPC], dt.float32, tag="red", name="red")
    c1 = float(np.float32(6.28125))
    c2 = float(np.float32(TWO_PI - 6.28125))
    c3 = float(np.float32(TWO_PI - 6.28125 - float(np.float32(TWO_PI - 6.28125))))
    nc.vector.cody_waite_cascade(red[:], ang[:], kf[:], c1, c2, c3)
    te0 = cst.tile([TIME, GPC], dt.float32, tag="te0", name="te0")
    nc.scalar.activation(te0[:], red[:], AF.Sin)
    p_t1 = psum([2 * TIME, GPC])
    nc.tensor.matmul(p_t1[:], tw1[:], te0[:])
    s_t1 = cst.tile([2 * TIME, GPC], dt.float32, tag="s_t1", name="s_t1")
    nc.scalar.activation(s_t1[:], p_t1[:], AF.Silu, bias=Bv["tb1"])
    p_t2 = psum([TIME, GPC])
    nc.tensor.matmul(p_t2[:], tw2[:], s_t1[:])
    teT = cst.tile([TIME, GPC], dt.float32r, tag="teT", name="teT")
    nc.vector.tensor_scalar(teT[:], p_t2[:], Bv["tb2"], None, op0=AluOpType.add)

    te_eb, te_nb = [], []
    for l in range(L):
        p_b = psum([NODE, GPC])
        nc.tensor.matmul(p_b[:], W[("ew1d", l)], teT[:])
        eb = cst.tile([NODE, GPC], dt.float32, tag=f"te_eb{l}", name=f"te_eb{l}")
        nc.vector.tensor_scalar(eb[:], p_b[:], Bv[("eb1", l)], None, op0=AluOpType.add)
        te_eb.append(eb)
        p_b2 = psum([NODE, GPC])
        nc.tensor.matmul(p_b2[:], W[("nw1d", l)], teT[:])
        nbt = cst.tile([NODE, GPC], dt.float32, tag=f"te_nb{l}", name=f"te_nb{l}")
        nc.vector.tensor_scalar(nbt[:], p_b2[:], Bv[("nb1", l)], None, op0=AluOpType.add)
        te_nb.append(nbt)

    # ---- node embedding (zT staged through G's SBUF space, overwritten later) ----
    zT = G[:, 0:NPC]
    nc.sync.dma_start(zT, aps["zT"][:])
    for c in range(NPC // CH):
        p_h = psum([NODE, CH])
        nc.tensor.matmul(p_h[:], embw[:], zT[:, c * CH:(c + 1) * CH])
        nc.vector.tensor_scalar(hT[:, c * CH:(c + 1) * CH], p_h[:], Bv["embb"],
                                None, op0=AluOpType.add)

    # ---- graph build (software-pipelined over graphs) ----
    GS = [dict() for _ in range(GPC)]

    def b0(g):
        st = GS[g]
        cg = cartT[:, g * NPG:(g + 1) * NPG]
        c2t = wrk.tile([3, NPG], dt.float32, tag="c2t", name="c2t", bufs=2)
        nc.scalar.square(c2t[:], cg)
        c3x2 = wrk.tile([3, NPG], dt.float32, tag="c3x2", name="c3x2", bufs=2)
        nc.scalar.mul(c3x2[:], cg, 2.0)
        p_sq = psum([1, NPG], tag="p_sq", bufs=1)
        nc.tensor.matmul(p_sq[:], ones3[:], c2t[:])
        sqr = wrk.tile([1, NPG], dt.float32, tag="sqr", name="sqr", bufs=2)
        nc.vector.tensor_copy(sqr[:], p_sq[:])
        nsqr = wrk.tile([1, NPG], dt.float32, tag="nsqr", name="nsqr", bufs=2)
        nc.scalar.mul(nsqr[:], sqr[:], -1.0)
        negr = wrk.tile([1, NPG], dt.float32, tag="negr", name="negr", bufs=2)
        nc.scalar.mul(negr[:], onesr, -1.0)
        p_D = psum([NPG, NPG], tag="p_D", bufs=2)
        nc.tensor.matmul(p_D[:], c3x2[:], cg, start=True, stop=False)
        nc.tensor.matmul(p_D[:], negr[:], sqr[:], start=False, stop=False)
        nc.tensor.matmul(p_D[:], nsqr[:], onesr, start=False, stop=True)
        Bt = wrk.tile([NPG, NPG], dt.float32, tag="Bt", name="Bt", bufs=2)
        nc.vector.tensor_add(Bt[:], p_D[:], negdiag[:])
        st["Bt"] = Bt

    def b1(g):
        st = GS[g]
        Bt = st.pop("Bt")
        mx1 = wrk.tile([NPG, 8], dt.float32, tag="mx1", name="mx1", bufs=2)
        ix1 = wrk.tile([NPG, 8], dt.uint32, tag="ix1", name="ix1", bufs=2)
        nc.vector.max(mx1[:], Bt[:])
        nc.vector.max_index(ix1[:], mx1[:], Bt[:])
        B2t = wrk.tile([NPG, NPG], dt.float32, tag="B2t", name="B2t", bufs=2)
        nc.vector.match_replace(B2t[:], mx1[:], Bt[:], NEG_BIG)
        mx2 = wrk.tile([NPG, 8], dt.float32, tag="mx2", name="mx2", bufs=2)
        ix2 = wrk.tile([NPG, 8], dt.uint32, tag="ix2", name="ix2", bufs=2)
        nc.vector.max(mx2[:], B2t[:])
        nc.vector.max_index(ix2[:], mx2[:], B2t[:])
        idxf = wrk.tile([NPG, KNN], dt.float32r, tag="idxf", name="idxf", bufs=3)
        nc.vector.tensor_copy(idxf[:, 0:8], ix1[:])
        nc.vector.tensor_copy(idxf[:, 8:12], ix2[:, 0:4])
        dnm = wrk.tile([NPG, KNN], dt.float32r, tag="dnm", name="dnm", bufs=3)
        nc.scalar.mul(dnm[:, 0:8], mx1[:], -1.0)
        nc.scalar.mul(dnm[:, 8:12], mx2[:, 0:4], -1.0)
        st["idxf"], st["dnm"] = idxf, dnm

    def b2(g):
        st = GS[g]
        scr_i = drp.tile([NPG, KNN], dt.float32r, tag="scr_i", name="scr_i", bufs=3)
        scr_d = drp.tile([NPG, KNN], dt.float32r, tag="scr_d", name="scr_d", bufs=3)
        nc.sync.dma_start(scr_i[:], st.pop("idxf")[:])
        nc.scalar.dma_start(scr_d[:], st.pop("dnm")[:])
        idx_row = wrk.tile([1, EPG], dt.float32r, tag="idx_row", name="idx_row", bufs=2)
        nc.sync.dma_start(idx_row[:], scr_i[:].rearrange("p k -> (p k)")[None, :])
        nc.scalar.dma_start(dist_slice(g, 0, EPG),
                            scr_d[:].rearrange("p k -> (p k)")[None, :])
        st["idx_row"] = idx_row

    def b3(g):
        st = GS[g]
        idx_row = st.pop("idx_row")
        for c in range(NCH):
            e0 = g * EPG + c * CH
            p_bc = psum([NPG, CH])
            nc.tensor.matmul(p_bc[:], onesr_r[:], idx_row[:, c * CH:(c + 1) * CH])
            nc.vector.tensor_scalar(G[:, e0:e0 + CH], p_bc[:], pidxc, None,
                                    op0=AluOpType.is_equal)

    def b4(g):
        j = 32 * (g // GPR)
        for c in range(NCH):
            e0 = g * EPG + c * CH
            p_df = psum([3, CH])
            nc.tensor.matmul(p_df[:], chi_s(0, g), G[:, e0:e0 + CH],
                             start=True, stop=False)
            nc.tensor.matmul(p_df[:], chi_s(1, g), G[:, e0:e0 + CH],
                             start=False, stop=False)
            nc.tensor.matmul(p_df[:], chi_s(2, g), gdst[:, c * CH:(c + 1) * CH],
                             start=False, stop=False)
            nc.tensor.matmul(p_df[:], chi_s(3, g), gdst[:, c * CH:(c + 1) * CH],
                             start=False, stop=True)
            p_d3 = psum([3, CH])
            nc.tensor.matmul(p_d3[:], ones13r2[j:j + 1, :], dist_slice(g, c * CH, CH))
            d3 = wrk.tile([3, CH], dt.float32, tag="d3", name="d3", bufs=2)
            nc.scalar.activation(d3[:], p_d3[:], AF.Sqrt, bias=Bv["sqrt_eps"])
            r3 = wrk.tile([3, CH], dt.float32, tag="r3", name="r3", bufs=2)
            nc.vector.reciprocal(r3[:], d3[:])
            nc.vector.tensor_tensor(udiff[:, e0:e0 + CH], p_df[:], r3[:],
                                    op=AluOpType.mult)

    bstages = [b0, b1, b2, b3, b4]
    for i in range(GPC + len(bstages) - 1):
        for sidx, fn in enumerate(bstages):
            k = i - sidx
            if 0 <= k < GPC:
                fn(k)

    # ---- layers (software-pipelined over 512-edge units) ----
    for l in range(L):
        pgs, qgs = [], []
        for g in range(GPC):
            hg = hT[:, g * NPG:(g + 1) * NPG]
            p_p = psum([NPG, NODE])
            nc.tensor.matmul(p_p[:], hg, W[("ew1a", l)])
            pg = wrk.tile([NPG, NODE], dt.float32r, tag="pg", name="pg", bufs=GPC + 1)
            nc.vector.tensor_copy(pg[:], p_p[:])
            p_q = psum([NPG, NODE])
            nc.tensor.matmul(p_q[:], hg, W[("ew1b", l)])
            qg = wrk.tile([NPG, NODE], dt.float32r, tag="qg", name="qg", bufs=GPC + 1)
            nc.vector.tensor_copy(qg[:], p_q[:])
            pgs.append(pg); qgs.append(qg)

        units = [(g, c) for g in range(GPC) for c in range(NCH)]
        S = {}
        mTs, csTs = {}, {}

        def stage0(u, l=l, pgs=pgs, qgs=qgs, S=S):
            g, c = u
            j = 32 * (g // GPR)
            e0 = g * EPG + c * CH
            p_e = psum([NODE, CH])
            nc.tensor.matmul(p_e[:], pgs[g][:], G[:, e0:e0 + CH],
                             start=True, stop=False)
            nc.tensor.matmul(p_e[:], qgs[g][:], gdst[:, c * CH:(c + 1) * CH],
                             start=False, stop=False)
            nc.tensor.matmul(p_e[:], ew1c2(l)[j:j + 1, :],
                             dist_slice(g, c * CH, CH), start=False, stop=True)
            h1 = wrk.tile([NODE, CH], dt.float32r, tag="h1", name="h1", bufs=3)
            nc.scalar.activation(h1[:], p_e[:], AF.Silu, bias=te_eb[l][:, g:g + 1])
            S[u] = {"h1": h1}

        def stage1(u, l=l, S=S, mTs=mTs):
            g, c = u
            if c == 0:
                mTs[g] = wrk.tile([NODE, EPG], dt.float32r, tag="mT", name="mT", bufs=2)
            p_m = psum([NODE, CH])
            nc.tensor.matmul(p_m[:], W[("ew2", l)], S[u]["h1"][:])
            nc.scalar.activation(mTs[g][:, c * CH:(c + 1) * CH], p_m[:], AF.Silu,
                                 bias=Bv[("eb2", l)])

        def stage2(u, l=l, S=S, mTs=mTs):
            g, c = u
            p_c = psum([NODE, CH])
            nc.tensor.matmul(p_c[:], W[("cw1", l)], mTs[g][:, c * CH:(c + 1) * CH])
            ch1 = wrk.tile([NODE, CH], dt.float32r, tag="ch1", name="ch1", bufs=3)
            nc.scalar.activation(ch1[:], p_c[:], AF.Silu, bias=Bv[("cb1", l)])
            S[u]["ch1"] = ch1

        def stage3(u, l=l, S=S, mTs=mTs, csTs=csTs):
            g, c = u
            e0 = g * EPG + c * CH
            if c == 0:
                csTs[g] = wrk.tile([3, EPG], dt.float32, tag="csT", name="csT", bufs=2)
            p_g3 = psum([3, CH])
            nc.tensor.matmul(p_g3[:], W[("cw2r3", l)], S[u]["ch1"][:])
            nc.vector.tensor_tensor(csTs[g][:, c * CH:(c + 1) * CH], p_g3[:],
                                    udiff[:, e0:e0 + CH], op=AluOpType.mult)
            S.pop(u)
            if c == NCH - 1:
                nc.vector.tensor_reduce(miT[:, g * NPG:(g + 1) * NPG],
                                        mTs[g][:].rearrange("p (n k) -> p n k", k=KNN),
                                        axis=mybir.AxisListType.X, op=AluOpType.add)
                ctmp = wrk.tile([3, NPG], dt.float32, tag="ctmp", name="ctmp", bufs=2)
                nc.vector.tensor_reduce(ctmp[:],
                                        csTs[g][:].rearrange("p (n k) -> p n k", k=KNN),
                                        axis=mybir.AxisListType.X, op=AluOpType.add)
                nc.vector.tensor_add(coordT[:, g * NPG:(g + 1) * NPG],
                                     coordT[:, g * NPG:(g + 1) * NPG], ctmp[:])

        stages = [stage0, stage1, stage2, stage3]
        n = len(units)
        for i in range(n + len(stages) - 1):
            for sidx, fn in enumerate(stages):
                k = i - sidx
                if 0 <= k < n:
                    fn(units[k])

        for c in range(NPC // CH):
            n0 = c * CH
            p_n = psum([NODE, CH])
            nc.tensor.matmul(p_n[:], W[("nw1a", l)], hT[:, n0:n0 + CH],
                             start=True, stop=False)
            nc.tensor.matmul(p_n[:], W[("nw1b", l)], miT[:, n0:n0 + CH],
                             start=False, stop=True)
            sn = wrk.tile([NODE, CH], dt.float32r, tag="sn", name="sn", bufs=2)
            for gg in range(CH // NPG):
                g = (n0 + gg * NPG) // NPG
                nc.scalar.activation(sn[:, gg * NPG:(gg + 1) * NPG],
                                     p_n[:, gg * NPG:(gg + 1) * NPG], AF.Silu,
                                     bias=te_nb[l][:, g:g + 1])
            p_n2 = psum([NODE, CH])
            nc.tensor.matmul(p_n2[:], W[("nw2", l)], sn[:])
            nc.vector.scalar_tensor_tensor(hT[:, n0:n0 + CH], p_n2[:], Bv[("nb2", l)],
                                           hT[:, n0:n0 + CH],
                                           op0=AluOpType.add, op1=AluOpType.add)

    nc.sync.dma_start(aps["o_coordT"][:], coordT[:])
    nc.sync.dma_start(aps["o_hT"][:], hT[:])
    if _CACHE.get("debug"):
        nc.sync.dma_start(aps["o_udiff"][:], udiff[:])
        nc.sync.dma_start(aps["o_miT"][:], miT[:])


def _build():
    if "nc" in _CACHE:
        return _CACHE["nc"]
    nc = bacc.Bacc("TRN2", target_bir_lowering=False, debug=False,
                   enable_asserts=True, num_devices=N_CORES)
    aps = _declare(nc)
    with tile.TileContext(nc, trace_sim=False) as tc:
        with ExitStack() as ctx:
            with nc.allow_low_precision(reason="fp32r storage is fp32-width"):
                _emit(nc, tc, ctx, aps)
    nc.compile()
    _CACHE["nc"] = nc
    return nc


def _host_inputs(z_nodes, t, cart_coords, batch_indices, params):
    z = np.asarray(z_nodes, np.float32)
    tt = np.asarray(t, np.float32)
    cart = np.asarray(cart_coords, np.float32)

    half = TIME // 2
    freqs = np.exp(np.arange(half, dtype=np.float32) * (-math.log(10000.0) / (half - 1)))

    wcat = np.zeros((NODE, len(WNAMES) * L * NODE + 3 * L), np.float32)
    for l, lp in enumerate(params["layers"]):
        ew1 = np.asarray(lp["edge_w1"], np.float32)
        nw1 = np.asarray(lp["node_w1"], np.float32)
        mats = {
            "ew1a": ew1[0:NODE], "ew1b": ew1[NODE:2 * NODE], "ew1d": ew1[2 * NODE + 1:],
            "ew2": np.asarray(lp["edge_w2"], np.float32),
            "cw1": np.asarray(lp["coord_w1"], np.float32),
            "nw1a": nw1[0:NODE], "nw1b": nw1[NODE:2 * NODE], "nw1d": nw1[2 * NODE:],
            "nw2": np.asarray(lp["node_w2"], np.float32),
        }
        for i, nm in enumerate(WNAMES):
            c0 = (l * len(WNAMES) + i) * NODE
            wcat[:, c0:c0 + NODE] = mats[nm]
    c0 = len(WNAMES) * L * NODE
    for l, lp in enumerate(params["layers"]):
        cw2 = np.asarray(lp["coord_w2"], np.float32)
        wcat[:, c0 + 3 * l:c0 + 3 * l + 3] = np.repeat(cw2, 3, axis=1)

    bcat = np.zeros((ZDIM, len(BNAMES) * L + 5), np.float32)
    for l, lp in enumerate(params["layers"]):
        vals = {
            "eb1": lp["edge_b1"], "eb2": lp["edge_b2"], "cb1": lp["coord_b1"],
            "nb1": lp["node_b1"], "nb2": lp["node_b2"],
        }
        for i, nm in enumerate(BNAMES):
            bcat[0:NODE, l * len(BNAMES) + i] = np.asarray(vals[nm], np.float32)
    nbi = len(BNAMES) * L
    bcat[0:NODE, nbi] = np.asarray(params["emb_b"], np.float32)
    bcat[0:NODE, nbi + 1] = np.asarray(params["time_b2"], np.float32)
    bcat[:, nbi + 2] = np.asarray(params["time_b1"], np.float32)
    bcat[:, nbi + 3] = np.arange(ZDIM, dtype=np.float32)
    bcat[0:3, nbi + 4] = 1e-8

    e13 = np.zeros((33, L * NODE + 3), np.float32)
    for l, lp in enumerate(params["layers"]):
        w1c = np.asarray(lp["edge_w1"], np.float32)[2 * NODE]
        e13[0, l * NODE:(l + 1) * NODE] = w1c
        e13[32, l * NODE:(l + 1) * NODE] = w1c
    e13[0, L * NODE:] = 1.0
    e13[32, L * NODE:] = 1.0

    rowcat_shared = np.zeros((1, 2 * TIME + NPG + 2 * GPC), np.float32)
    rowcat_shared[0, 0:half] = freqs
    rowcat_shared[0, half:TIME] = freqs
    rowcat_shared[0, TIME + half:2 * TIME] = np.pi / 2
    rowcat_shared[0, 2 * TIME:2 * TIME + NPG] = 1.0
    rowcat_shared[0, 2 * TIME + NPG + GPC:] = 1.0

    shared = {
        "wcat": wcat,
        "bcat": bcat,
        "e13": e13,
        "onesr_r": np.ones((1, NPG), np.float32),
        "ones3": np.ones((3, 1), np.float32),
        "negdiag": (np.eye(NPG) * NEG_BIG).astype(np.float32),
        "gdst": np.kron(np.eye(NPG, dtype=np.float32), np.ones((1, KNN), np.float32)),
        "tw1": np.asarray(params["time_w1"], np.float32),
        "tw2": np.asarray(params["time_w2"], np.float32),
        "embw": np.asarray(params["emb_w"], np.float32),
    }

    in_maps = []
    for cix in range(N_CORES):
        n0 = cix * NPC
        zc = z[n0:n0 + NPC]
        cc = cart[n0:n0 + NPC]
        cnm = np.zeros((NPG, 3 * GPC), np.float32)
        for g in range(GPC):
            cnm[:, 3 * g:3 * g + 3] = cc[g * NPG:(g + 1) * NPG]
        cnm_hi = (cnm.view(np.uint32) & np.uint32(0xFFFF0000)).view(np.float32)
        cnm_lo = cnm - cnm_hi
        rc = rowcat_shared.copy()
        rc[0, 2 * TIME + NPG:2 * TIME + NPG + GPC] = tt[cix * GPC:(cix + 1) * GPC]
        m = dict(shared)
        m["rowcat"] = rc
        m["zT"] = np.ascontiguousarray(zc.T)
        m["cartT"] = np.ascontiguousarray(cc.T)
        m["chi_cat"] = np.concatenate([cnm_hi, cnm_lo, -cnm_hi, -cnm_lo], axis=1)
        in_maps.append(m)
    return in_maps


def kernel(z_nodes, t, cart_coords, batch_indices, params):
    nc = _build()
    in_maps = _host_inputs(z_nodes, t, cart_coords, batch_indices, params)
    res = run_bass_kernel_spmd(nc, in_maps, list(range(N_CORES))).results
    coord = np.concatenate([np.ascontiguousarray(r["o_coordT"].T) for r in res], axis=0)
    h = np.concatenate([np.ascontiguousarray(r["o_hT"].T) for r in res], axis=0)
    return coord.astype(np.float32), h.astype(np.float32)
